# revision 45
# baseline (speedup 1.0000x reference)
"""Trainium2 Bass kernel for EquivariantMultiheadAttention (sparse attention).

Problem shapes: b=4, n=512, c=256, h=8, d=32, dg=6, hid=16.

Strategy (8 NeuronCores, no collectives):
  - Shard (batch b x n-half): core i handles b = i//2, query rows
    n0 = 256*(i%2) .. n0+256.  Keys/values replicated per batch; keys with
    mask=False are dropped on the host (exact: softmax weight 0) and padded
    to m_pad with a -100 bias column (exp underflows to 0 in the
    max-free softmax; |pre| <= ~10 so no overflow).
  - The kernel is ACT(scalar-engine)-bound: the two Silu passes over the
    location-MLP hidden layers cost 0.833ns/col + 185ns fixed per
    instruction.  P1 is built around MERGED silu instructions: one ACT op
    reads [l1-preact(chunk s) | l2-preact(chunk s-2)] as a single
    [128, 4banks, 3rows, 136] = 1632-col PSUM access pattern.
  - PSUM: two persistent 4-bank tiles (PP0/PP1), ping-pong by chunk
    parity.  A chunk = 6 query rows x one M-half (136 cols); rows sit in
    136-col slots, 3 per bank (sub-tile range tracking pipelines disjoint
    slots freely).  l3 reuses a just-read l2 slot; A_feat lives in the
    104-f32 spare tail of each bank (<=104-col pieces); Q/K/V projections
    borrow slots mid-pipeline under scheduler wait_until gates.
  - Biases: b1 rides a 7th ones-channel of gt (w1 row 6 = b1); b2 is one
    [1x128] bias matmul per l2 row; b3 is the silu3 activation bias.
  - All MLP matmuls are f16 (1.0 PE cycles/row at any p-state, ~1e-3 rel).
  - l3 accumulates per-chunk into one [128,136] slot (rows are
    partition-disjoint: partitions 8*(r%16)), then DVE assembles into the
    SBUF accumulator acc_all; silu3 runs batched over 4 groups.
  - Softmax: exp per group with fused row sums (f16 att), reciprocal on
    DVE, normalization scale on the idle GPSIMD, f16 PE transposes (f16
    identity) writing two groups into one psum bank so attT copies move
    [msz,2,128] per instruction.  AV + output projection + out DMA run
    per output half so the post-exp tail stays short.
  - Critical DMAs (gt0, w1, w2, b2, b3) ride the HWDGE (sync) queue in
    need order; bulk constants ride the Pool SWDGE queue.
"""
import sys

sys.path.insert(0, "/opt/trn_rl_repo")

import numpy as np
import concourse.bacc as bacc
import concourse.mybir as mybir
import concourse.tile as tile
from concourse.bass_utils import run_bass_kernel_spmd

F32 = mybir.dt.float32
F32R = mybir.dt.float32r
F16 = mybir.dt.float16
AF = mybir.ActivationFunctionType
AX = mybir.AxisListType
ALU = mybir.AluOpType

B, N, C, H, DG, HID = 4, 512, 256, 8, 6, 16
DG1 = DG + 1        # gt gets a 7th ones-channel carrying b1
D = C // H          # 32
NP = N // 2         # 256 query rows per core
NG = NP // 16       # 16 groups of 16 rows
AV_DT = F16


def _build(nc_mod, m_pad):
    """Emit the SPMD single-core program. m_pad: compacted+padded key count."""
    nc = nc_mod
    M = m_pad
    MH = M // 2         # half-M chunk columns (136 for m_pad=272)
    assert 3 * MH <= 512
    MT = [(t, min(128, M - 128 * t)) for t in range((M + 127) // 128)]
    SPARE = 512 - 3 * MH        # per-bank spare f32 tail (104)
    # A_feat spare-slot pieces covering M in <=SPARE-col chunks
    AFC = []
    pos = 0
    while pos < M:
        sz = min(SPARE, M - pos)
        AFC.append((pos, sz))
        pos += sz

    # ---------------- I/O ----------------
    gt = nc.declare_dram_parameter("gt", [DG1, NP, M], F16, isOutput=False)
    ctq = nc.declare_dram_parameter("ctq", [C, NP], F32, isOutput=False)
    ctk = nc.declare_dram_parameter("ctk", [C, M], F32, isOutput=False)
    wq = nc.declare_dram_parameter("wq", [C, C], F32, isOutput=False)
    wk = nc.declare_dram_parameter("wk", [C, C], F32, isOutput=False)
    win = nc.declare_dram_parameter("win", [C, C], F32, isOutput=False)
    wout = nc.declare_dram_parameter("wout", [C, C], F32, isOutput=False)
    bq = nc.declare_dram_parameter("bq", [1, C], F32, isOutput=False)
    bk = nc.declare_dram_parameter("bk", [1, C], F32, isOutput=False)
    bin_ = nc.declare_dram_parameter("bin", [1, C], F32, isOutput=False)
    bout = nc.declare_dram_parameter("bout", [1, C], F32, isOutput=False)
    w1 = nc.declare_dram_parameter("w1", [DG1, 128], F16, isOutput=False)
    w2 = nc.declare_dram_parameter("w2", [128, 128], F16, isOutput=False)
    w3 = nc.declare_dram_parameter("w3", [128, 8], F16, isOutput=False)
    b2 = nc.declare_dram_parameter("b2", [1, 128], F16, isOutput=False)
    b3 = nc.declare_dram_parameter("b3", [128, 1], F32, isOutput=False)
    mb = nc.declare_dram_parameter("mb", [1, M], F16, isOutput=False)
    ident = nc.declare_dram_parameter("ident", [128, 128], F16, isOutput=False)
    out = nc.declare_dram_parameter("out", [NP, C], F32, isOutput=True)

    # chunk schedule: rblocks of 6 rows (last one 4), x 2 halves
    RB = []
    r0 = 0
    while r0 < NP:
        RB.append((r0, min(6, NP - r0)))
        r0 += 6
    chunks = []
    for (r0, nr) in RB:
        for hf in range(2):
            chunks.append((r0, nr, hf))
    NC = len(chunks)

    with tile.TileContext(nc) as tc:
        import contextlib
        with contextlib.ExitStack() as ctx:
            cst = ctx.enter_context(tc.tile_pool(name="cst", bufs=1))
            big = ctx.enter_context(tc.tile_pool(name="big", bufs=1))
            gtp = ctx.enter_context(tc.tile_pool(name="gtp", bufs=3))
            xp = ctx.enter_context(tc.tile_pool(name="xp", bufs=4))
            smp = ctx.enter_context(tc.tile_pool(name="smp", bufs=6))
            ppp = ctx.enter_context(tc.tile_pool(name="ppp", bufs=1,
                                                 space="PSUM"))

            # two persistent 4-bank psum tiles; all psum is hand-placed
            # sub-views of these (sub-tile range tracking orders access).
            PP = [ppp.tile([128, 4, 512], F32, tag=f"pp{i}", name=f"pp{i}")
                  for i in range(2)]

            def slot(pp, j, sz=None):
                # row slot j (bank j//3, 136-col slot j%3)
                return pp[:, j // 3, MH * (j % 3):MH * (j % 3) + (sz or MH)]

            def spare(pp, bk_, sz):
                return pp[:, bk_, 3 * MH:3 * MH + sz]

            # ---- critical-path DMAs on HWDGE (sync) in need order ----
            gt_tiles = {}

            def need_gt(g):
                gt_t = gtp.tile([DG1, 16, M], F16, tag="gt", name=f"gt{g}")
                nc.sync.dma_start(out=gt_t,
                                  in_=gt[:, 16 * g:16 * (g + 1), :])
                gt_tiles[g] = gt_t

            need_gt(0)
            w1_sb = cst.tile([DG1, 128], F16, tag="w1")
            nc.sync.dma_start(out=w1_sb, in_=w1[:, :])
            w2_sb = cst.tile([128, 128], F16, tag="w2")
            nc.sync.dma_start(out=w2_sb, in_=w2[:, :])
            b2_sb = cst.tile([1, 128], F16, tag="b2")
            nc.sync.dma_start(out=b2_sb, in_=b2[:, :])
            b3_sb = cst.tile([128, 1], F32, tag="b3")
            nc.sync.dma_start(out=b3_sb, in_=b3[:, :])
            # f32 ones row + f16 ones row (dtype-converting copy)
            onr = cst.tile([1, 512], F32, tag="onr")
            nc.vector.memset(onr, 1.0)
            onh = cst.tile([1, 3 * MH], F16, tag="onh")
            nc.vector.tensor_copy(out=onh, in_=onr[:, :3 * MH])
            misc = {"onr": onr}

            # ---- deferred-prologue emitters ----
            w3_sb = [cst.tile([128, 128], F16, tag=f"w3{j}", name=f"w3{j}")
                     for j in range(16)]
            qt_sb = [big.tile([128, NP], F16, tag=f"qt{i}", name=f"qt{i}")
                     for i in range(2)]
            kt_sb = [big.tile([128, M], F16, tag=f"kt{i}", name=f"kt{i}")
                     for i in range(2)]
            v_sb = [big.tile([128, C], AV_DT, tag=f"v{t}", name=f"v{t}")
                    for t, _ in MT]
            qm_all = big.tile([128, 2, NG, 16, H], F16, tag="qm")
            wq_sb, wk_sb, win_sb, wout_sb = [], [], [], []
            ctq_sb, ctk_sb = [], []

            def emit_w3_dmas():
                zs = cst.tile([128, 128], F32, tag="zs", name="zs")
                nc.vector.memset(zs, 0.0)
                w3c = cst.tile([128, 8], F16, tag="w3c", name="w3c")
                nc.gpsimd.dma_start(out=w3c, in_=w3[:, :])
                for j in range(16):
                    nc.vector.tensor_copy(out=w3_sb[j], in_=zs)
                    nc.vector.tensor_copy(out=w3_sb[j][:, 8 * j:8 * j + 8],
                                          in_=w3c)

            def emit_qk_consts():
                for ci in range(2):
                    t = cst.tile([128, C], F32, tag=f"wq{ci}", name=f"wq{ci}")
                    nc.gpsimd.dma_start(out=t, in_=wq[128 * ci:128 * (ci + 1), :])
                    wq_sb.append(t)
                    t = cst.tile([128, C], F32, tag=f"wk{ci}", name=f"wk{ci}")
                    nc.gpsimd.dma_start(out=t, in_=wk[128 * ci:128 * (ci + 1), :])
                    wk_sb.append(t)
                    t = cst.tile([128, NP], F32, tag=f"cq{ci}", name=f"cq{ci}")
                    nc.gpsimd.dma_start(out=t, in_=ctq[128 * ci:128 * (ci + 1), :])
                    ctq_sb.append(t)
                    t = cst.tile([128, M], F32, tag=f"ck{ci}", name=f"ck{ci}")
                    nc.gpsimd.dma_start(out=t, in_=ctk[128 * ci:128 * (ci + 1), :])
                    ctk_sb.append(t)
                for nm, src in (("bq", bq), ("bk", bk)):
                    t = cst.tile([1, C], F32, tag=nm, name=nm)
                    nc.gpsimd.dma_start(out=t, in_=src[:, :])
                    misc[nm] = t
                t = cst.tile([1, M], F16, tag="mb", name="mbt")
                nc.gpsimd.dma_start(out=t, in_=mb[:, :])
                misc["mb"] = t
                t = cst.tile([1, 128], F32, tag="onc", name="onct")
                nc.vector.memset(t, 1.0)
                misc["onc"] = t

            def proj_T(dst, w_tiles, b_row, rhs_tiles, nfree, pp):
                # borrows bank-2/3 row slots of `pp`; range deps order it
                # within the pipeline.
                for ct in range(2):
                    p = pp[:, 2 + ct, 0:nfree]
                    for ci in range(2):
                        nc.tensor.matmul(
                            p,
                            w_tiles[ci][:, 128 * ct:128 * (ct + 1)],
                            rhs_tiles[ci][:, :nfree],
                            start=(ci == 0), stop=False)
                    nc.tensor.matmul(
                        p, b_row[:, 128 * ct:128 * (ct + 1)],
                        misc["onr"][:, :nfree], start=False, stop=True)
                    nc.vector.tensor_copy(out=dst[ct], in_=p)

            def emit_qt():
                proj_T(qt_sb, wq_sb, misc["bq"], ctq_sb, NP, PP[0])
                # pack Q into the per-c-half A_feat lhsT layout (block
                # sparse: head h only on its own 32 c-rows): zero-fill,
                # then one strided copy per head
                nc.vector.memset(qm_all.bitcast(F32), 0.0)
                for h in range(H):
                    a = 32 * (h % 4)
                    nc.vector.tensor_copy(
                        out=qm_all[a:a + 32, h // 4, :, :, h],
                        in_=qt_sb[h // 4][a:a + 32, :].rearrange(
                            "p (g q) -> p g q", g=NG))

            def emit_kt():
                proj_T(kt_sb, wk_sb, misc["bk"], ctk_sb, M, PP[1])

            def emit_v_consts():
                for ci in range(2):
                    t = cst.tile([128, C], F32, tag=f"wi{ci}", name=f"wi{ci}")
                    nc.gpsimd.dma_start(out=t, in_=win[128 * ci:128 * (ci + 1), :])
                    win_sb.append(t)
                t = cst.tile([1, C], F32, tag="bi", name="bi")
                nc.gpsimd.dma_start(out=t, in_=bin_[:, :])
                misc["bi"] = t

            def emit_v():
                for t_, msz in MT:
                    p = PP[1][:, t_ % 2, 0:C]
                    for ci in range(2):
                        nc.tensor.matmul(
                            p[:msz, :],
                            ctk_sb[ci][:, 128 * t_:128 * t_ + msz],
                            win_sb[ci], start=(ci == 0), stop=False)
                    nc.tensor.matmul(
                        p[:msz, :], misc["onc"][:, :msz],
                        misc["bi"], start=False, stop=True)
                    nc.vector.tensor_copy(out=v_sb[t_][:msz, :],
                                          in_=p[:msz, :])

            def emit_tail_consts():
                t = cst.tile([128, 128], F16, tag="id", name="idt")
                nc.gpsimd.dma_start(out=t, in_=ident[:, :])
                misc["id"] = t
                for ci in range(2):
                    t = cst.tile([128, C], F32R, tag=f"wo{ci}", name=f"wo{ci}")
                    nc.gpsimd.dma_start(
                        out=t,
                        in_=wout[128 * ci:128 * (ci + 1), :].bitcast(F32R))
                    wout_sb.append(t)
                t = cst.tile([1, C], F32, tag="bo", name="bo")
                nc.gpsimd.dma_start(out=t, in_=bout[:, :])
                misc["bo"] = t

            # ---------------- P1 ----------------
            af_sb = big.tile([128, NG, M], F32, tag="af")
            acc_all = big.tile([128, NG, M], F32, tag="acc")
            pre_all = big.tile([128, NG, M], F32, tag="pre")
            x1x2 = {}
            acc_first = set()

            def emit_af(g):
                # A_feat for group g in the spare tails of PP[g % 2]
                pp = PP[g % 2]
                for i, (mp, sz) in enumerate(AFC):
                    ps = spare(pp, i, sz)
                    nc.tensor.matmul(ps, qm_all[:, 0, g, :, :],
                                     kt_sb[0][:, mp:mp + sz],
                                     start=True, stop=False)
                    nc.tensor.matmul(ps, qm_all[:, 1, g, :, :],
                                     kt_sb[1][:, mp:mp + sz],
                                     start=False, stop=False)
                    nc.tensor.matmul(ps, onh[:, :128],
                                     misc["mb"][:, mp:mp + sz],
                                     start=False, stop=True)
                    nc.vector.tensor_copy(out=af_sb[:, g, mp:mp + sz],
                                          in_=ps)

            def emit_silu3(q):
                # groups 4q..4q+4: swish(acc + b3), then + af -> pre
                al = smp.tile([128, 4, M], F32, tag="aloc", name=f"aloc{q}")
                nc.scalar.activation(out=al,
                                     in_=acc_all[:, 4 * q:4 * q + 4, :],
                                     func=AF.Silu, bias=b3_sb, scale=1.0)
                nc.vector.tensor_add(pre_all[:, 4 * q:4 * q + 4, :], al,
                                     af_sb[:, 4 * q:4 * q + 4, :])

            deferred = {
                0: [(emit_w3_dmas, None)],
                2: [(emit_qk_consts, None)],
                6: [(emit_qt, 0.012)],
                8: [(emit_kt, 0.014)],
                10: [(emit_v_consts, 0.016)],
                12: [(emit_v, 0.018)],
                14: [(emit_tail_consts, 0.020)],
            }
            af_at = {12 + 2 * g: g for g in range(NG)}
            s3_at = {}
            for q in range(4):
                last_row = 16 * (4 * q + 4) - 1
                for ci_, (r0, nr, hf) in enumerate(chunks):
                    if r0 <= last_row < r0 + nr and hf == 1:
                        s3_at[ci_ + 2] = q

            for s in range(NC + 2):
                for fn, gate in deferred.get(s, ()):
                    if gate is None:
                        fn()
                    else:
                        with tc.tile_wait_until(gate):
                            fn()
                g_af = af_at.get(s)
                if g_af is not None:
                    emit_af(g_af)

                ppa = PP[s % 2]
                if s < NC:
                    r0, nr, hf = chunks[s]
                    # prefetch one group ahead so l1 never stalls on the
                    # ~1.4us HWDGE latency at group boundaries
                    for g_need in (r0 // 16, (r0 + nr - 1) // 16,
                                   min(NG - 1, (r0 + nr - 1) // 16 + 1)):
                        if g_need not in gt_tiles:
                            need_gt(g_need)
                    # l1 rows into slots 0..nr (banks 0-1); b1 rides the
                    # gt ones-channel folded into w1
                    for j in range(nr):
                        r = r0 + j
                        nc.tensor.matmul(
                            slot(ppa, j),
                            w1_sb,
                            gt_tiles[r // 16][:, r % 16,
                                              MH * hf:MH * hf + MH],
                            start=True, stop=True)
                if s - 2 >= 0 and s - 2 < NC:
                    k = s - 2
                    r0k, nrk, hfk = chunks[k]
                    xk = x1x2[k]
                    # l2(k) into slots 6..6+nr (banks 2-3) + b2 bias matmul
                    for j in range(nrk):
                        nc.tensor.matmul(
                            slot(ppa, 6 + j),
                            w2_sb, xk[:, j // 3, j % 3, :],
                            start=True, stop=False)
                        nc.tensor.matmul(
                            slot(ppa, 6 + j),
                            b2_sb, onh[:, :MH],
                            start=False, stop=True)
                # merged silu: [l1(s) | l2(s-2)] in one 1632-col AP
                xt = xp.tile([128, 4, 3, MH], F16, tag="x", name=f"x{s}")
                if s < NC and s - 2 >= 0:
                    nc.scalar.activation(
                        out=xt,
                        in_=ppa[:, :, 0:3 * MH].rearrange(
                            "p b (r m) -> p b r m", r=3),
                        func=AF.Silu, scale=1.0)
                elif s < NC:
                    nc.scalar.activation(
                        out=xt[:, 0:2, :, :],
                        in_=ppa[:, 0:2, 0:3 * MH].rearrange(
                            "p b (r m) -> p b r m", r=3),
                        func=AF.Silu, scale=1.0)
                else:
                    nc.scalar.activation(
                        out=xt[:, 2:4, :, :],
                        in_=ppa[:, 2:4, 0:3 * MH].rearrange(
                            "p b (r m) -> p b r m", r=3),
                        func=AF.Silu, scale=1.0)
                x1x2[s] = xt
                # l3(s-2): consume x2(s-2) = x1x2[s][:, 2:4]; accumulate
                # into bank2-slot0 (partition-disjoint rows), then DVE
                # assembles into acc_all
                if s - 2 >= 0 and s - 2 < NC:
                    k = s - 2
                    r0k, nrk, hfk = chunks[k]
                    # split rows at group boundaries; each run accumulates
                    # in its own slot (w3 variants write exact zeros on all
                    # other partitions, so full-partition DVE adds are safe)
                    runs = []
                    rcur = r0k
                    while rcur < r0k + nrk:
                        g = rcur // 16
                        rend = min(r0k + nrk, 16 * (g + 1))
                        runs.append((g, rcur, rend))
                        rcur = rend
                    for i, (g, ra, rb) in enumerate(runs):
                        l3ps = slot(ppa, 6 + i)
                        for jj, r in enumerate(range(ra, rb)):
                            j = r - r0k
                            nc.tensor.matmul(
                                l3ps, w3_sb[r % 16],
                                x1x2[s][:, 2 + j // 3, j % 3, :],
                                start=(jj == 0), stop=(jj == rb - ra - 1))
                        dst = acc_all[:, g, MH * hfk:MH * hfk + MH]
                        if (g, hfk) not in acc_first:
                            acc_first.add((g, hfk))
                            nc.vector.tensor_copy(out=dst, in_=l3ps)
                        else:
                            nc.vector.tensor_add(dst, dst, l3ps)
                    del x1x2[k]
                q3 = s3_at.get(s)
                if q3 is not None:
                    emit_silu3(q3)

            # ------------- P2: softmax + transpose; P3 per half ---------
            attT = big.tile([128, len(MT), 16, 16, 8], AV_DT, tag="attT")
            avf = smp.tile([128, 2, 256], F32R, tag="avf")

            def emit_av_half(hff):
                pavs = [PP[0][:, t, :].rearrange("p (a b) -> p a b", a=2)
                        for t, _ in MT]
                for h in range(H):
                    for t, msz in MT:
                        nc.tensor.matmul(
                            pavs[t][32 * (h % 4):32 * (h % 4) + 32, h // 4,
                                    128 * hff:128 * hff + 128],
                            v_sb[t][:msz, 32 * h:32 * h + 32],
                            attT[:msz, t, 8 * hff:8 * hff + 8, :, h],
                            start=True, stop=True,
                            tile_position=(0, 32 * (h % 4)))
                sl = np.s_[:, :, 128 * hff:128 * hff + 128]
                acc = smp.tile([128, 2, 128], F32, tag="avacc",
                               name=f"avacc{hff}")
                nc.vector.tensor_copy(out=acc, in_=pavs[0][sl])
                for t in range(1, len(MT) - 1):
                    nc.vector.tensor_add(acc, acc, pavs[t][sl])
                nc.vector.tensor_add(avf[sl], acc, pavs[len(MT) - 1][sl])
                po = PP[1][:, 0, 0:C]
                for ci in range(2):
                    nc.tensor.matmul(
                        po,
                        avf[:, ci, 128 * hff:128 * hff + 128],
                        wout_sb[ci], start=(ci == 0), stop=False)
                nc.tensor.matmul(po, misc["onc"],
                                 misc["bo"], start=False, stop=True)
                o_sb = smp.tile([128, C], F32, tag="osb", name=f"osb{hff}")
                nc.vector.tensor_copy(out=o_sb, in_=po)
                nc.sync.dma_start(out=out[128 * hff:128 * (hff + 1), :],
                                  in_=o_sb)

            for gp in range(NG // 2):
                # f16 view of bank 3 of the alternating PP tile: two
                # groups' transposes share it so attT copies move
                # [msz, 2, 128] per instruction.
                pTv = PP[gp % 2][:, 3, :].bitcast(F16).rearrange(
                    "p (a b) -> p a b", a=2)
                for j in range(2):
                    g = 2 * gp + j
                    att = smp.tile([128, M], F16, tag="att", name=f"att{g}")
                    sm = smp.tile([128, 1], F32, tag="sm", name=f"sm{g}")
                    nc.scalar.activation(out=att, in_=pre_all[:, g, :],
                                         func=AF.Exp, scale=1.0,
                                         accum_out=sm)
                    rc = smp.tile([128, 1], F32, tag="rc", name=f"rc{g}")
                    nc.vector.reciprocal(out=rc, in_=sm)
                    atts = smp.tile([128, M], F16, tag="atts",
                                    name=f"atts{g}")
                    eng = nc.vector if g >= NG - 2 else nc.gpsimd
                    eng.tensor_scalar_mul(atts, att, rc)
                    for t, msz in MT:
                        nc.tensor.transpose(
                            pTv[:msz, j, 128 * t:128 * t + 128],
                            atts[:, 128 * t:128 * t + msz],
                            misc["id"])
                for t, msz in MT:
                    nc.vector.tensor_copy(
                        out=attT[:msz, t, 2 * gp:2 * gp + 2, :, :],
                        in_=pTv[:msz, :, 128 * t:128 * t + 128])
                if gp == NG // 4 - 1:
                    emit_av_half(0)
            emit_av_half(1)

    nc.finalize()
    return nc


_CACHE = {}


def _get_nc(m_pad):
    if m_pad not in _CACHE:
        _CACHE[m_pad] = _build(bacc.Bacc(None, target_bir_lowering=False), m_pad)
    return _CACHE[m_pad]


def prepare(inputs):
    """Host-side sharding/packing. Returns (nc, in_maps, assemble)."""
    pg = np.asarray(inputs["pairwise_g"], np.float32)
    cf = np.asarray(inputs["coset_functions"], np.float32)
    mask = np.asarray(inputs["mask"])
    idxs = [np.where(mask[b])[0] for b in range(B)]
    maxc = max(len(ix) for ix in idxs)
    m_pad = max(256, -(-maxc // 16) * 16)

    lb1 = np.asarray(inputs["loc_b1"], np.float32).reshape(128)
    w1a = np.zeros((DG1, 128), np.float32)
    w1a[:DG] = np.asarray(inputs["loc_w1"], np.float32).transpose(1, 0, 2) \
        .reshape(DG, 128)
    w1a[DG] = lb1                       # b1 rides the gt ones-channel
    w2b = np.zeros((128, 128), np.float32)
    lw2 = np.asarray(inputs["loc_w2"], np.float32)
    for h in range(H):
        w2b[16 * h:16 * (h + 1), 16 * h:16 * (h + 1)] = lw2[h]
    lw3 = np.asarray(inputs["loc_w3"], np.float32)
    w3p = np.zeros((128, 8), np.float32)
    for h in range(H):
        w3p[16 * h:16 * (h + 1), h] = lw3[h, :, 0]
    b2v = np.asarray(inputs["loc_b2"], np.float32).reshape(1, 128)
    b3v = np.tile(np.asarray(inputs["loc_b3"], np.float32).reshape(8), 16)
    b3v = b3v.reshape(128, 1)

    common = {
        "wq": np.asarray(inputs["fc_q_w"], np.float32) / np.float32(16.0),
        "wk": np.asarray(inputs["fc_k_w"], np.float32),
        "win": np.asarray(inputs["in_w"], np.float32),
        "wout": np.asarray(inputs["out_w"], np.float32),
        "bq": (np.asarray(inputs["fc_q_b"], np.float32) / np.float32(16.0)
               ).reshape(1, C),
        "bk": np.asarray(inputs["fc_k_b"], np.float32).reshape(1, C),
        "bin": np.asarray(inputs["in_b"], np.float32).reshape(1, C),
        "bout": np.asarray(inputs["out_b"], np.float32).reshape(1, C),
        "w1": w1a.astype(np.float16), "w2": w2b.astype(np.float16),
        "w3": w3p.astype(np.float16),
        "b2": b2v.astype(np.float16), "b3": b3v,
        "ident": np.eye(128, dtype=np.float16),
    }
    common = {k: np.ascontiguousarray(v) for k, v in common.items()}

    in_maps = []
    for core in range(8):
        b, nh = core // 2, core % 2
        ix = idxs[b]
        cnt = len(ix)
        n0 = NP * nh
        gtc = np.zeros((DG1, NP, m_pad), np.float16)
        gtc[:DG, :, :cnt] = pg[b, n0:n0 + NP][:, ix, :].transpose(2, 0, 1)
        gtc[DG, :, :] = np.float16(1.0)   # ones-channel carrying b1
        ctk = np.zeros((C, m_pad), np.float32)
        ctk[:, :cnt] = cf[b, ix, :].T
        # pads at -100: exp underflows to 0 in the max-free softmax
        mbv = np.zeros((1, m_pad), np.float16)
        mbv[0, cnt:] = np.float16(-100.0)
        im = dict(common)
        im["gt"] = np.ascontiguousarray(gtc)
        im["ctq"] = np.ascontiguousarray(cf[b, n0:n0 + NP, :].T)
        im["ctk"] = np.ascontiguousarray(ctk)
        im["mb"] = mbv
        in_maps.append(im)

    def assemble(results):
        o = np.empty((B, N, C), np.float32)
        for core in range(8):
            b, nh = core // 2, core % 2
            o[b, NP * nh:NP * (nh + 1), :] = results[core]["out"]
        return o

    return _get_nc(m_pad), in_maps, assemble


def kernel(**inputs) -> np.ndarray:
    nc, in_maps, assemble = prepare(inputs)
    res = run_bass_kernel_spmd(nc, in_maps, list(range(8)))
    return assemble(res.results)


# revision 48
# speedup vs baseline: 1.0040x; 1.0040x over previous
"""Trainium2 Bass kernel for EquivariantMultiheadAttention (sparse attention).

Problem shapes: b=4, n=512, c=256, h=8, d=32, dg=6, hid=16.

Strategy (8 NeuronCores, no collectives):
  - Shard (batch b x n-half): core i handles b = i//2, query rows
    n0 = 256*(i%2) .. n0+256.  Keys/values replicated per batch; keys with
    mask=False are dropped on the host (exact: softmax weight 0) and padded
    to m_pad with a -100 bias column (exp underflows to 0 in the
    max-free softmax; |pre| <= ~10 so no overflow).
  - The kernel is ACT(scalar-engine)-bound: the two Silu passes over the
    location-MLP hidden layers cost 0.833ns/col + 185ns fixed per
    instruction.  P1 is built around MERGED silu instructions: one ACT op
    reads [l1-preact(chunk s) | l2-preact(chunk s-2)] as a single
    [128, 4banks, 3rows, 136] = 1632-col PSUM access pattern.
  - PSUM: two persistent 4-bank tiles (PP0/PP1), ping-pong by chunk
    parity.  A chunk = 6 query rows x one M-half (136 cols); rows sit in
    136-col slots, 3 per bank (sub-tile range tracking pipelines disjoint
    slots freely).  l3 reuses a just-read l2 slot; A_feat lives in the
    104-f32 spare tail of each bank (<=104-col pieces); Q/K/V projections
    borrow slots mid-pipeline under scheduler wait_until gates.
  - Biases: b1 rides a 7th ones-channel of gt (w1 row 6 = b1); b2 is one
    [1x128] bias matmul per l2 row; b3 is the silu3 activation bias.
  - All MLP matmuls are f16 (1.0 PE cycles/row at any p-state, ~1e-3 rel).
  - l3 accumulates per-chunk into one [128,136] slot (rows are
    partition-disjoint: partitions 8*(r%16)), then DVE assembles into the
    SBUF accumulator acc_all; silu3 runs batched over 4 groups.
  - Softmax: exp per group with fused row sums (f16 att), reciprocal on
    DVE, normalization scale on the idle GPSIMD, f16 PE transposes (f16
    identity) writing two groups into one psum bank so attT copies move
    [msz,2,128] per instruction.  AV + output projection + out DMA run
    per output half so the post-exp tail stays short.
  - Critical DMAs (gt0, w1, w2, b2, b3) ride the HWDGE (sync) queue in
    need order; bulk constants ride the Pool SWDGE queue.
"""
import sys

sys.path.insert(0, "/opt/trn_rl_repo")

import numpy as np
import concourse.bacc as bacc
import concourse.mybir as mybir
import concourse.tile as tile
from concourse.bass_utils import run_bass_kernel_spmd

F32 = mybir.dt.float32
F32R = mybir.dt.float32r
F16 = mybir.dt.float16
AF = mybir.ActivationFunctionType
AX = mybir.AxisListType
ALU = mybir.AluOpType

B, N, C, H, DG, HID = 4, 512, 256, 8, 6, 16
DG1 = DG + 1        # gt gets a 7th ones-channel carrying b1
D = C // H          # 32
NP = N // 2         # 256 query rows per core
NG = NP // 16       # 16 groups of 16 rows
AV_DT = F16


def _build(nc_mod, m_pad):
    """Emit the SPMD single-core program. m_pad: compacted+padded key count."""
    nc = nc_mod
    M = m_pad
    MH = M // 2         # half-M chunk columns (136 for m_pad=272)
    assert 3 * MH <= 512
    MT = [(t, min(128, M - 128 * t)) for t in range((M + 127) // 128)]
    SPARE = 512 - 3 * MH        # per-bank spare f32 tail (104)
    # A_feat spare-slot pieces covering M in <=SPARE-col chunks
    AFC = []
    pos = 0
    while pos < M:
        sz = min(SPARE, M - pos)
        AFC.append((pos, sz))
        pos += sz

    # ---------------- I/O ----------------
    gt = nc.declare_dram_parameter("gt", [DG1, NP, M], F16, isOutput=False)
    ctq = nc.declare_dram_parameter("ctq", [C, NP], F32, isOutput=False)
    ctk = nc.declare_dram_parameter("ctk", [C, M], F32, isOutput=False)
    wq = nc.declare_dram_parameter("wq", [C, C], F32, isOutput=False)
    wk = nc.declare_dram_parameter("wk", [C, C], F32, isOutput=False)
    win = nc.declare_dram_parameter("win", [C, C], F32, isOutput=False)
    wout = nc.declare_dram_parameter("wout", [C, C], F32, isOutput=False)
    bq = nc.declare_dram_parameter("bq", [1, C], F32, isOutput=False)
    bk = nc.declare_dram_parameter("bk", [1, C], F32, isOutput=False)
    bin_ = nc.declare_dram_parameter("bin", [1, C], F32, isOutput=False)
    bout = nc.declare_dram_parameter("bout", [1, C], F32, isOutput=False)
    w1 = nc.declare_dram_parameter("w1", [DG1, 128], F16, isOutput=False)
    w2 = nc.declare_dram_parameter("w2", [128, 128], F16, isOutput=False)
    w3 = nc.declare_dram_parameter("w3", [128, 8], F16, isOutput=False)
    b2 = nc.declare_dram_parameter("b2", [1, 128], F16, isOutput=False)
    b3 = nc.declare_dram_parameter("b3", [128, 1], F32, isOutput=False)
    mb = nc.declare_dram_parameter("mb", [1, M], F16, isOutput=False)
    ident = nc.declare_dram_parameter("ident", [128, 128], F16, isOutput=False)
    out = nc.declare_dram_parameter("out", [NP, C], F32, isOutput=True)

    # chunk schedule: rblocks of 6 rows (last one 4), x 2 halves
    RB = []
    r0 = 0
    while r0 < NP:
        RB.append((r0, min(6, NP - r0)))
        r0 += 6
    chunks = []
    for (r0, nr) in RB:
        for hf in range(2):
            chunks.append((r0, nr, hf))
    NC = len(chunks)

    with tile.TileContext(nc) as tc:
        import contextlib
        with contextlib.ExitStack() as ctx:
            cst = ctx.enter_context(tc.tile_pool(name="cst", bufs=1))
            big = ctx.enter_context(tc.tile_pool(name="big", bufs=1))
            gtp = ctx.enter_context(tc.tile_pool(name="gtp", bufs=3))
            xp = ctx.enter_context(tc.tile_pool(name="xp", bufs=4))
            smp = ctx.enter_context(tc.tile_pool(name="smp", bufs=6))
            ppp = ctx.enter_context(tc.tile_pool(name="ppp", bufs=1,
                                                 space="PSUM"))

            # two persistent 4-bank psum tiles; all psum is hand-placed
            # sub-views of these (sub-tile range tracking orders access).
            PP = [ppp.tile([128, 4, 512], F32, tag=f"pp{i}", name=f"pp{i}")
                  for i in range(2)]

            def slot(pp, j, sz=None):
                # row slot j (bank j//3, 136-col slot j%3)
                return pp[:, j // 3, MH * (j % 3):MH * (j % 3) + (sz or MH)]

            def spare(pp, bk_, sz):
                return pp[:, bk_, 3 * MH:3 * MH + sz]

            # ---- critical-path DMAs on HWDGE (sync) in need order ----
            gt_tiles = {}

            def need_gt(g):
                gt_t = gtp.tile([DG1, 16, M], F16, tag="gt", name=f"gt{g}")
                nc.sync.dma_start(out=gt_t,
                                  in_=gt[:, 16 * g:16 * (g + 1), :])
                gt_tiles[g] = gt_t

            need_gt(0)
            w1_sb = cst.tile([DG1, 128], F16, tag="w1")
            nc.sync.dma_start(out=w1_sb, in_=w1[:, :])
            w2_sb = cst.tile([128, 128], F16, tag="w2")
            nc.sync.dma_start(out=w2_sb, in_=w2[:, :])
            b2_sb = cst.tile([1, 128], F16, tag="b2")
            nc.sync.dma_start(out=b2_sb, in_=b2[:, :])
            b3_sb = cst.tile([128, 1], F32, tag="b3")
            nc.sync.dma_start(out=b3_sb, in_=b3[:, :])
            # f32 ones row + f16 ones row (dtype-converting copy)
            onr = cst.tile([1, 512], F32, tag="onr")
            nc.vector.memset(onr, 1.0)
            onh = cst.tile([1, 3 * MH], F16, tag="onh")
            nc.vector.tensor_copy(out=onh, in_=onr[:, :3 * MH])
            misc = {"onr": onr}

            # ---- deferred-prologue emitters ----
            w3_sb = [cst.tile([128, 128], F16, tag=f"w3{j}", name=f"w3{j}")
                     for j in range(16)]
            qt_sb = [big.tile([128, NP], F16, tag=f"qt{i}", name=f"qt{i}")
                     for i in range(2)]
            kt_sb = [big.tile([128, M], F16, tag=f"kt{i}", name=f"kt{i}")
                     for i in range(2)]
            v_sb = [big.tile([128, C], AV_DT, tag=f"v{t}", name=f"v{t}")
                    for t, _ in MT]
            qm_all = big.tile([128, 2, NG, 16, H], F16, tag="qm")
            wq_sb, wk_sb, win_sb, wout_sb = [], [], [], []
            ctq_sb, ctk_sb = [], []

            def emit_w3_dmas():
                zs = cst.tile([128, 128], F32, tag="zs", name="zs")
                nc.vector.memset(zs, 0.0)
                w3c = cst.tile([128, 8], F16, tag="w3c", name="w3c")
                nc.gpsimd.dma_start(out=w3c, in_=w3[:, :])
                for j in range(16):
                    nc.vector.tensor_copy(out=w3_sb[j], in_=zs)
                    nc.vector.tensor_copy(out=w3_sb[j][:, 8 * j:8 * j + 8],
                                          in_=w3c)

            def emit_qk_consts():
                for ci in range(2):
                    t = cst.tile([128, C], F32, tag=f"wq{ci}", name=f"wq{ci}")
                    nc.gpsimd.dma_start(out=t, in_=wq[128 * ci:128 * (ci + 1), :])
                    wq_sb.append(t)
                    t = cst.tile([128, C], F32, tag=f"wk{ci}", name=f"wk{ci}")
                    nc.gpsimd.dma_start(out=t, in_=wk[128 * ci:128 * (ci + 1), :])
                    wk_sb.append(t)
                    t = cst.tile([128, NP], F32, tag=f"cq{ci}", name=f"cq{ci}")
                    nc.gpsimd.dma_start(out=t, in_=ctq[128 * ci:128 * (ci + 1), :])
                    ctq_sb.append(t)
                    t = cst.tile([128, M], F32, tag=f"ck{ci}", name=f"ck{ci}")
                    nc.gpsimd.dma_start(out=t, in_=ctk[128 * ci:128 * (ci + 1), :])
                    ctk_sb.append(t)
                for nm, src in (("bq", bq), ("bk", bk)):
                    t = cst.tile([1, C], F32, tag=nm, name=nm)
                    nc.gpsimd.dma_start(out=t, in_=src[:, :])
                    misc[nm] = t
                t = cst.tile([1, M], F16, tag="mb", name="mbt")
                nc.gpsimd.dma_start(out=t, in_=mb[:, :])
                misc["mb"] = t
                t = cst.tile([1, 128], F32, tag="onc", name="onct")
                nc.vector.memset(t, 1.0)
                misc["onc"] = t

            def proj_T(dst, w_tiles, b_row, rhs_tiles, nfree, pp):
                # borrows bank-2/3 row slots of `pp`; range deps order it
                # within the pipeline.
                for ct in range(2):
                    p = pp[:, 2 + ct, 0:nfree]
                    for ci in range(2):
                        nc.tensor.matmul(
                            p,
                            w_tiles[ci][:, 128 * ct:128 * (ct + 1)],
                            rhs_tiles[ci][:, :nfree],
                            start=(ci == 0), stop=False)
                    nc.tensor.matmul(
                        p, b_row[:, 128 * ct:128 * (ct + 1)],
                        misc["onr"][:, :nfree], start=False, stop=True)
                    nc.vector.tensor_copy(out=dst[ct], in_=p)

            def emit_qt():
                proj_T(qt_sb, wq_sb, misc["bq"], ctq_sb, NP, PP[0])
                # pack Q into the per-c-half A_feat lhsT layout (block
                # sparse: head h only on its own 32 c-rows): zero-fill,
                # then one strided copy per head
                nc.vector.memset(qm_all.bitcast(F32), 0.0)
                for h in range(H):
                    a = 32 * (h % 4)
                    nc.vector.tensor_copy(
                        out=qm_all[a:a + 32, h // 4, :, :, h],
                        in_=qt_sb[h // 4][a:a + 32, :].rearrange(
                            "p (g q) -> p g q", g=NG))

            def emit_kt():
                proj_T(kt_sb, wk_sb, misc["bk"], ctk_sb, M, PP[1])

            def emit_v_consts():
                for ci in range(2):
                    t = cst.tile([128, C], F32, tag=f"wi{ci}", name=f"wi{ci}")
                    nc.gpsimd.dma_start(out=t, in_=win[128 * ci:128 * (ci + 1), :])
                    win_sb.append(t)
                t = cst.tile([1, C], F32, tag="bi", name="bi")
                nc.gpsimd.dma_start(out=t, in_=bin_[:, :])
                misc["bi"] = t

            def emit_v():
                for t_, msz in MT:
                    p = PP[1][:, t_ % 2, 0:C]
                    for ci in range(2):
                        nc.tensor.matmul(
                            p[:msz, :],
                            ctk_sb[ci][:, 128 * t_:128 * t_ + msz],
                            win_sb[ci], start=(ci == 0), stop=False)
                    nc.tensor.matmul(
                        p[:msz, :], misc["onc"][:, :msz],
                        misc["bi"], start=False, stop=True)
                    nc.vector.tensor_copy(out=v_sb[t_][:msz, :],
                                          in_=p[:msz, :])

            def emit_tail_consts():
                t = cst.tile([128, 128], F16, tag="id", name="idt")
                nc.gpsimd.dma_start(out=t, in_=ident[:, :])
                misc["id"] = t
                for ci in range(2):
                    t = cst.tile([128, C], F32R, tag=f"wo{ci}", name=f"wo{ci}")
                    nc.gpsimd.dma_start(
                        out=t,
                        in_=wout[128 * ci:128 * (ci + 1), :].bitcast(F32R))
                    wout_sb.append(t)
                t = cst.tile([1, C], F32, tag="bo", name="bo")
                nc.gpsimd.dma_start(out=t, in_=bout[:, :])
                misc["bo"] = t

            # ---------------- P1 ----------------
            af_sb = big.tile([128, NG, M], F32, tag="af")
            acc_all = big.tile([128, NG, M], F32, tag="acc")
            pre_all = big.tile([128, NG, M], F32, tag="pre")
            x1x2 = {}
            acc_first = set()

            def emit_af(g):
                # A_feat for group g in the spare tails of PP[g % 2]
                pp = PP[g % 2]
                for i, (mp, sz) in enumerate(AFC):
                    ps = spare(pp, i, sz)
                    nc.tensor.matmul(ps, qm_all[:, 0, g, :, :],
                                     kt_sb[0][:, mp:mp + sz],
                                     start=True, stop=False)
                    nc.tensor.matmul(ps, qm_all[:, 1, g, :, :],
                                     kt_sb[1][:, mp:mp + sz],
                                     start=False, stop=False)
                    nc.tensor.matmul(ps, onh[:, :128],
                                     misc["mb"][:, mp:mp + sz],
                                     start=False, stop=True)
                    nc.vector.tensor_copy(out=af_sb[:, g, mp:mp + sz],
                                          in_=ps)

            def emit_silu3(q):
                # groups 4q..4q+4: swish(acc + b3), then + af -> pre
                al = smp.tile([128, 4, M], F32, tag="aloc", name=f"aloc{q}")
                nc.scalar.activation(out=al,
                                     in_=acc_all[:, 4 * q:4 * q + 4, :],
                                     func=AF.Silu, bias=b3_sb, scale=1.0)
                nc.vector.tensor_add(pre_all[:, 4 * q:4 * q + 4, :], al,
                                     af_sb[:, 4 * q:4 * q + 4, :])

            deferred = {
                0: [(emit_w3_dmas, None)],
                2: [(emit_qk_consts, None)],
                6: [(emit_qt, 0.012)],
                8: [(emit_kt, 0.014)],
                10: [(emit_v_consts, 0.016)],
                12: [(emit_v, 0.018)],
                14: [(emit_tail_consts, 0.020)],
            }
            af_at = {12 + 2 * g: g for g in range(NG)}
            s3_at = {}
            for q in range(4):
                last_row = 16 * (4 * q + 4) - 1
                for ci_, (r0, nr, hf) in enumerate(chunks):
                    if r0 <= last_row < r0 + nr and hf == 1:
                        s3_at[ci_ + 4] = q

            for s in range(NC + 4):
                for fn, gate in deferred.get(s, ()):
                    if gate is None:
                        fn()
                    else:
                        with tc.tile_wait_until(gate):
                            fn()
                g_af = af_at.get(s)
                if g_af is not None:
                    emit_af(g_af)

                ppa = PP[s % 2]
                ppb = PP[(s + 1) % 2]
                if s < NC:
                    r0, nr, hf = chunks[s]
                    # prefetch one group ahead so l1 never stalls on the
                    # ~1.4us HWDGE latency at group boundaries
                    for g_need in (r0 // 16, (r0 + nr - 1) // 16,
                                   min(NG - 1, (r0 + nr - 1) // 16 + 1)):
                        if g_need not in gt_tiles:
                            need_gt(g_need)
                    # l1 rows into slots 0..nr (banks 0-1); b1 rides the
                    # gt ones-channel folded into w1
                    for j in range(nr):
                        r = r0 + j
                        nc.tensor.matmul(
                            slot(ppa, j),
                            w1_sb,
                            gt_tiles[r // 16][:, r % 16,
                                              MH * hf:MH * hf + MH],
                            start=True, stop=True)
                if 0 <= s - 2 < NC:
                    k = s - 2
                    r0k, nrk, hfk = chunks[k]
                    xk = x1x2[k]
                    # l2(k) into slots 6..6+nr (banks 2-3) + b2 bias matmul
                    for j in range(nrk):
                        nc.tensor.matmul(
                            slot(ppa, 6 + j),
                            w2_sb, xk[:, j // 3, j % 3, :],
                            start=True, stop=False)
                        nc.tensor.matmul(
                            slot(ppa, 6 + j),
                            b2_sb, onh[:, :MH],
                            start=False, stop=True)
                # l3(s-4): two steps of slack so the sem-counter order
                # [.., l3(s-4), merged(s), l1(s+1), l2(s-1), ..] never
                # chains merged(s+1) behind merged(s).  Uses the OTHER
                # tile's l2 slots (read by merged(s-1), rewritten by
                # l2(s-1) at step s+1 after the DVE assembly read).
                if 0 <= s - 4 < NC:
                    k = s - 4
                    r0k, nrk, hfk = chunks[k]
                    runs = []
                    rcur = r0k
                    while rcur < r0k + nrk:
                        g = rcur // 16
                        rend = min(r0k + nrk, 16 * (g + 1))
                        runs.append((g, rcur, rend))
                        rcur = rend
                    for i, (g, ra, rb) in enumerate(runs):
                        l3ps = slot(ppb, 6 + i)
                        for jj, r in enumerate(range(ra, rb)):
                            j = r - r0k
                            nc.tensor.matmul(
                                l3ps, w3_sb[r % 16],
                                x1x2[k + 2][:, 2 + j // 3, j % 3, :],
                                start=(jj == 0), stop=(jj == rb - ra - 1))
                        dst = acc_all[:, g, MH * hfk:MH * hfk + MH]
                        if (g, hfk) not in acc_first:
                            acc_first.add((g, hfk))
                            nc.vector.tensor_copy(out=dst, in_=l3ps)
                        else:
                            nc.vector.tensor_add(dst, dst, l3ps)
                    x1x2.pop(k + 2, None)
                # merged silu: [l1(s) | l2(s-2)] in one 1632-col AP
                if s < NC + 2:
                    xt = xp.tile([128, 4, 3, MH], F16, tag="x", name=f"x{s}")
                    if s < NC and s - 2 >= 0:
                        nc.scalar.activation(
                            out=xt,
                            in_=ppa[:, :, 0:3 * MH].rearrange(
                                "p b (r m) -> p b r m", r=3),
                            func=AF.Silu, scale=1.0)
                    elif s < NC:
                        nc.scalar.activation(
                            out=xt[:, 0:2, :, :],
                            in_=ppa[:, 0:2, 0:3 * MH].rearrange(
                                "p b (r m) -> p b r m", r=3),
                            func=AF.Silu, scale=1.0)
                    else:
                        nc.scalar.activation(
                            out=xt[:, 2:4, :, :],
                            in_=ppa[:, 2:4, 0:3 * MH].rearrange(
                                "p b (r m) -> p b r m", r=3),
                            func=AF.Silu, scale=1.0)
                    x1x2[s] = xt
                q3 = s3_at.get(s)
                if q3 is not None:
                    emit_silu3(q3)

            # ------------- P2: softmax + transpose; P3 per half ---------
            attT = big.tile([128, len(MT), 16, 16, 8], AV_DT, tag="attT")
            avf = smp.tile([128, 2, 256], F32R, tag="avf")

            def emit_av_half(hff):
                pavs = [PP[0][:, t, :].rearrange("p (a b) -> p a b", a=2)
                        for t, _ in MT]
                for h in range(H):
                    for t, msz in MT:
                        nc.tensor.matmul(
                            pavs[t][32 * (h % 4):32 * (h % 4) + 32, h // 4,
                                    128 * hff:128 * hff + 128],
                            v_sb[t][:msz, 32 * h:32 * h + 32],
                            attT[:msz, t, 8 * hff:8 * hff + 8, :, h],
                            start=True, stop=True,
                            tile_position=(0, 32 * (h % 4)))
                sl = np.s_[:, :, 128 * hff:128 * hff + 128]
                acc = smp.tile([128, 2, 128], F32, tag="avacc",
                               name=f"avacc{hff}")
                nc.vector.tensor_copy(out=acc, in_=pavs[0][sl])
                for t in range(1, len(MT) - 1):
                    nc.vector.tensor_add(acc, acc, pavs[t][sl])
                nc.vector.tensor_add(avf[sl], acc, pavs[len(MT) - 1][sl])
                po = PP[1][:, 0, 0:C]
                for ci in range(2):
                    nc.tensor.matmul(
                        po,
                        avf[:, ci, 128 * hff:128 * hff + 128],
                        wout_sb[ci], start=(ci == 0), stop=False)
                nc.tensor.matmul(po, misc["onc"],
                                 misc["bo"], start=False, stop=True)
                o_sb = smp.tile([128, C], F32, tag="osb", name=f"osb{hff}")
                nc.vector.tensor_copy(out=o_sb, in_=po)
                nc.sync.dma_start(out=out[128 * hff:128 * (hff + 1), :],
                                  in_=o_sb)

            for gp in range(NG // 2):
                # f16 view of bank 3 of the alternating PP tile: two
                # groups' transposes share it so attT copies move
                # [msz, 2, 128] per instruction.
                pTv = PP[gp % 2][:, 3, :].bitcast(F16).rearrange(
                    "p (a b) -> p a b", a=2)
                for j in range(2):
                    g = 2 * gp + j
                    att = smp.tile([128, M], F16, tag="att", name=f"att{g}")
                    sm = smp.tile([128, 1], F32, tag="sm", name=f"sm{g}")
                    nc.scalar.activation(out=att, in_=pre_all[:, g, :],
                                         func=AF.Exp, scale=1.0,
                                         accum_out=sm)
                    rc = smp.tile([128, 1], F32, tag="rc", name=f"rc{g}")
                    nc.vector.reciprocal(out=rc, in_=sm)
                    atts = smp.tile([128, M], F16, tag="atts",
                                    name=f"atts{g}")
                    eng = nc.vector if g >= NG - 2 else nc.gpsimd
                    eng.tensor_scalar_mul(atts, att, rc)
                    for t, msz in MT:
                        nc.tensor.transpose(
                            pTv[:msz, j, 128 * t:128 * t + 128],
                            atts[:, 128 * t:128 * t + msz],
                            misc["id"])
                for t, msz in MT:
                    nc.vector.tensor_copy(
                        out=attT[:msz, t, 2 * gp:2 * gp + 2, :, :],
                        in_=pTv[:msz, :, 128 * t:128 * t + 128])
                if gp == NG // 4 - 1:
                    emit_av_half(0)
            emit_av_half(1)

    nc.finalize()
    return nc


_CACHE = {}


def _get_nc(m_pad):
    if m_pad not in _CACHE:
        _CACHE[m_pad] = _build(bacc.Bacc(None, target_bir_lowering=False), m_pad)
    return _CACHE[m_pad]


def prepare(inputs):
    """Host-side sharding/packing. Returns (nc, in_maps, assemble)."""
    pg = np.asarray(inputs["pairwise_g"], np.float32)
    cf = np.asarray(inputs["coset_functions"], np.float32)
    mask = np.asarray(inputs["mask"])
    idxs = [np.where(mask[b])[0] for b in range(B)]
    maxc = max(len(ix) for ix in idxs)
    m_pad = max(256, -(-maxc // 16) * 16)

    lb1 = np.asarray(inputs["loc_b1"], np.float32).reshape(128)
    w1a = np.zeros((DG1, 128), np.float32)
    w1a[:DG] = np.asarray(inputs["loc_w1"], np.float32).transpose(1, 0, 2) \
        .reshape(DG, 128)
    w1a[DG] = lb1                       # b1 rides the gt ones-channel
    w2b = np.zeros((128, 128), np.float32)
    lw2 = np.asarray(inputs["loc_w2"], np.float32)
    for h in range(H):
        w2b[16 * h:16 * (h + 1), 16 * h:16 * (h + 1)] = lw2[h]
    lw3 = np.asarray(inputs["loc_w3"], np.float32)
    w3p = np.zeros((128, 8), np.float32)
    for h in range(H):
        w3p[16 * h:16 * (h + 1), h] = lw3[h, :, 0]
    b2v = np.asarray(inputs["loc_b2"], np.float32).reshape(1, 128)
    b3v = np.tile(np.asarray(inputs["loc_b3"], np.float32).reshape(8), 16)
    b3v = b3v.reshape(128, 1)

    common = {
        "wq": np.asarray(inputs["fc_q_w"], np.float32) / np.float32(16.0),
        "wk": np.asarray(inputs["fc_k_w"], np.float32),
        "win": np.asarray(inputs["in_w"], np.float32),
        "wout": np.asarray(inputs["out_w"], np.float32),
        "bq": (np.asarray(inputs["fc_q_b"], np.float32) / np.float32(16.0)
               ).reshape(1, C),
        "bk": np.asarray(inputs["fc_k_b"], np.float32).reshape(1, C),
        "bin": np.asarray(inputs["in_b"], np.float32).reshape(1, C),
        "bout": np.asarray(inputs["out_b"], np.float32).reshape(1, C),
        "w1": w1a.astype(np.float16), "w2": w2b.astype(np.float16),
        "w3": w3p.astype(np.float16),
        "b2": b2v.astype(np.float16), "b3": b3v,
        "ident": np.eye(128, dtype=np.float16),
    }
    common = {k: np.ascontiguousarray(v) for k, v in common.items()}

    in_maps = []
    for core in range(8):
        b, nh = core // 2, core % 2
        ix = idxs[b]
        cnt = len(ix)
        n0 = NP * nh
        gtc = np.zeros((DG1, NP, m_pad), np.float16)
        gtc[:DG, :, :cnt] = pg[b, n0:n0 + NP][:, ix, :].transpose(2, 0, 1)
        gtc[DG, :, :] = np.float16(1.0)   # ones-channel carrying b1
        ctk = np.zeros((C, m_pad), np.float32)
        ctk[:, :cnt] = cf[b, ix, :].T
        # pads at -100: exp underflows to 0 in the max-free softmax
        mbv = np.zeros((1, m_pad), np.float16)
        mbv[0, cnt:] = np.float16(-100.0)
        im = dict(common)
        im["gt"] = np.ascontiguousarray(gtc)
        im["ctq"] = np.ascontiguousarray(cf[b, n0:n0 + NP, :].T)
        im["ctk"] = np.ascontiguousarray(ctk)
        im["mb"] = mbv
        in_maps.append(im)

    def assemble(results):
        o = np.empty((B, N, C), np.float32)
        for core in range(8):
            b, nh = core // 2, core % 2
            o[b, NP * nh:NP * (nh + 1), :] = results[core]["out"]
        return o

    return _get_nc(m_pad), in_maps, assemble


def kernel(**inputs) -> np.ndarray:
    nc, in_maps, assemble = prepare(inputs)
    res = run_bass_kernel_spmd(nc, in_maps, list(range(8)))
    return assemble(res.results)


# revision 61
# speedup vs baseline: 1.0586x; 1.0544x over previous
"""Trainium2 Bass kernel for EquivariantMultiheadAttention (sparse attention).

Problem shapes: b=4, n=512, c=256, h=8, d=32, dg=6, hid=16.

Strategy (8 NeuronCores, no collectives):
  - Shard (batch b x n-half): core i handles b = i//2, query rows
    n0 = 256*(i%2) .. n0+256.  Keys/values replicated per batch; keys with
    mask=False are dropped on the host (exact: softmax weight 0) and padded
    to m_pad with a -100 bias column (exp underflows to 0 in the
    max-free softmax; |pre| <= ~10 so no overflow).
  - The kernel is ACT(scalar-engine)-bound: the two Silu passes over the
    location-MLP hidden layers cost 0.833ns/col + 185ns fixed per
    instruction.  P1 is built around MERGED silu instructions: one ACT op
    reads [l1-preact(chunk s) | l2-preact(chunk s-2)] as a single
    [128, 4banks, 3rows, 136] = 1632-col PSUM access pattern.
  - PSUM: two persistent 4-bank tiles (PP0/PP1), ping-pong by chunk
    parity.  A chunk = 6 query rows x one M-half (136 cols); rows sit in
    136-col slots, 3 per bank (sub-tile range tracking pipelines disjoint
    slots freely).  l3 reuses a just-read l2 slot; A_feat lives in the
    104-f32 spare tail of each bank (<=104-col pieces); Q/K/V projections
    borrow slots mid-pipeline under scheduler wait_until gates.
  - Biases: b1 rides a 7th ones-channel of gt (w1 row 6 = b1); b2 is one
    [1x128] bias matmul per l2 row; b3 is the silu3 activation bias.
  - All MLP matmuls are f16 (1.0 PE cycles/row at any p-state, ~1e-3 rel).
  - l3 accumulates per-chunk into one [128,136] slot (rows are
    partition-disjoint: partitions 8*(r%16)), then DVE assembles into the
    SBUF accumulator acc_all; silu3 runs batched over 4 groups.
  - Softmax: exp per group with fused row sums (f16 att), reciprocal on
    DVE, normalization scale on the idle GPSIMD, f16 PE transposes (f16
    identity) writing two groups into one psum bank so attT copies move
    [msz,2,128] per instruction.  AV + output projection + out DMA run
    per output half so the post-exp tail stays short.
  - Critical DMAs (gt0, w1, w2, b2, b3) ride the HWDGE (sync) queue in
    need order; bulk constants ride the Pool SWDGE queue.
"""
import sys

sys.path.insert(0, "/opt/trn_rl_repo")

import numpy as np
import concourse.bacc as bacc
import concourse.mybir as mybir
import concourse.tile as tile
from concourse.bass_utils import run_bass_kernel_spmd

F32 = mybir.dt.float32
F32R = mybir.dt.float32r
F16 = mybir.dt.float16
AF = mybir.ActivationFunctionType
AX = mybir.AxisListType
ALU = mybir.AluOpType

B, N, C, H, DG, HID = 4, 512, 256, 8, 6, 16
DG1 = DG + 1        # gt gets a 7th ones-channel carrying b1
D = C // H          # 32
NP = N // 2         # 256 query rows per core
NG = NP // 16       # 16 groups of 16 rows
AV_DT = F16


def _build(nc_mod, m_pad):
    """Emit the SPMD single-core program. m_pad: compacted+padded key count."""
    nc = nc_mod
    M = m_pad
    MH = M // 2         # half-M chunk columns (136 for m_pad=272)
    assert 3 * MH <= 512
    MT = [(t, min(128, M - 128 * t)) for t in range((M + 127) // 128)]
    SPARE = 512 - 3 * MH        # per-bank spare f32 tail (104)
    # A_feat spare-slot pieces covering M in <=SPARE-col chunks
    AFC = []
    pos = 0
    while pos < M:
        sz = min(SPARE, M - pos)
        AFC.append((pos, sz))
        pos += sz

    # ---------------- I/O ----------------
    gt = nc.declare_dram_parameter("gt", [DG1, NP, M], F16, isOutput=False)
    ctq = nc.declare_dram_parameter("ctq", [C, NP], F32, isOutput=False)
    ctk = nc.declare_dram_parameter("ctk", [C, M], F32, isOutput=False)
    wq = nc.declare_dram_parameter("wq", [C, C], F32, isOutput=False)
    wk = nc.declare_dram_parameter("wk", [C, C], F32, isOutput=False)
    win = nc.declare_dram_parameter("win", [C, C], F32, isOutput=False)
    wout = nc.declare_dram_parameter("wout", [C, C], F32, isOutput=False)
    bq = nc.declare_dram_parameter("bq", [1, C], F32, isOutput=False)
    bk = nc.declare_dram_parameter("bk", [1, C], F32, isOutput=False)
    bin_ = nc.declare_dram_parameter("bin", [1, C], F32, isOutput=False)
    bout = nc.declare_dram_parameter("bout", [1, C], F32, isOutput=False)
    w1 = nc.declare_dram_parameter("w1", [DG1, 128], F16, isOutput=False)
    w2 = nc.declare_dram_parameter("w2", [128, 128], F16, isOutput=False)
    w3 = nc.declare_dram_parameter("w3", [128, 8], F16, isOutput=False)
    b2 = nc.declare_dram_parameter("b2", [1, 128], F16, isOutput=False)
    b3 = nc.declare_dram_parameter("b3", [128, 1], F32, isOutput=False)
    mb = nc.declare_dram_parameter("mb", [1, M], F16, isOutput=False)
    ident = nc.declare_dram_parameter("ident", [128, 128], F16, isOutput=False)
    out = nc.declare_dram_parameter("out", [NP, C], F32, isOutput=True)

    # chunk schedule: rblocks of 6 rows (last one 4), x 2 halves
    RB = []
    r0 = 0
    while r0 < NP:
        RB.append((r0, min(6, NP - r0)))
        r0 += 6
    chunks = []
    for (r0, nr) in RB:
        for hf in range(2):
            chunks.append((r0, nr, hf))
    NC = len(chunks)

    with tile.TileContext(nc) as tc:
        import contextlib
        with contextlib.ExitStack() as ctx:
            cst = ctx.enter_context(tc.tile_pool(name="cst", bufs=1))
            big = ctx.enter_context(tc.tile_pool(name="big", bufs=1))
            gtp = ctx.enter_context(tc.tile_pool(name="gtp", bufs=3))
            xp = ctx.enter_context(tc.tile_pool(name="xp", bufs=4))
            smp = ctx.enter_context(tc.tile_pool(name="smp", bufs=6))
            ppp = ctx.enter_context(tc.tile_pool(name="ppp", bufs=2,
                                                 space="PSUM"))

            # dependency tracking is per-TILE, so each step's 4-bank psum
            # is a fresh rotating pool tile: T[s] holds l1(s) (banks 0-1)
            # and l2(s-2) (banks 2-3); the merged silu reads the whole
            # tile.  l3(s-4) and A_feat borrow regions of T[s+1] before
            # l1(s+1)/l2(s-1) claim it (write->read->overwrite, tile-
            # fenced in emission order).
            T = {}

            def new_T(s):
                T[s] = ppp.tile([128, 4, 512], F32, tag="T", name=f"T{s}")
                return T[s]

            def slot(pp, j, sz=None):
                # row slot j (bank j//3, 136-col slot j%3)
                return pp[:, j // 3, MH * (j % 3):MH * (j % 3) + (sz or MH)]

            # ---- critical-path DMAs on HWDGE (sync) in need order ----
            gt_tiles = {}

            def need_gt(g):
                gt_t = gtp.tile([DG1, 16, M], F16, tag="gt", name=f"gt{g}")
                nc.sync.dma_start(out=gt_t,
                                  in_=gt[:, 16 * g:16 * (g + 1), :])
                gt_tiles[g] = gt_t

            need_gt(0)
            w1_sb = cst.tile([DG1, 128], F16, tag="w1")
            nc.sync.dma_start(out=w1_sb, in_=w1[:, :])
            w2_sb = cst.tile([128, 128], F16, tag="w2")
            nc.sync.dma_start(out=w2_sb, in_=w2[:, :])
            b2_sb = cst.tile([1, 128], F16, tag="b2")
            nc.sync.dma_start(out=b2_sb, in_=b2[:, :])
            b3_sb = cst.tile([128, 1], F32, tag="b3")
            nc.sync.dma_start(out=b3_sb, in_=b3[:, :])
            # f32 ones row + f16 ones row (dtype-converting copy)
            onr = cst.tile([1, 512], F32, tag="onr")
            nc.vector.memset(onr, 1.0)
            onh = cst.tile([1, 3 * MH], F16, tag="onh")
            nc.vector.tensor_copy(out=onh, in_=onr[:, :3 * MH])
            misc = {"onr": onr}

            # ---- deferred-prologue emitters ----
            w3_sb = [cst.tile([128, 128], F16, tag=f"w3{j}", name=f"w3{j}")
                     for j in range(16)]
            qt_sb = [big.tile([128, NP], F16, tag=f"qt{i}", name=f"qt{i}")
                     for i in range(2)]
            kt_sb = [big.tile([128, M], F16, tag=f"kt{i}", name=f"kt{i}")
                     for i in range(2)]
            v_sb = [big.tile([128, C], AV_DT, tag=f"v{t}", name=f"v{t}")
                    for t, _ in MT]
            qm_all = big.tile([128, 2, NG, 16, H], F16, tag="qm")
            wq_sb, wk_sb, win_sb, wout_sb = [], [], [], []
            ctq_sb, ctk_sb = [], []

            def emit_w3_dmas():
                zs = cst.tile([128, 128], F32, tag="zs", name="zs")
                nc.vector.memset(zs, 0.0)
                w3c = cst.tile([128, 8], F16, tag="w3c", name="w3c")
                nc.gpsimd.dma_start(out=w3c, in_=w3[:, :])
                for j in range(16):
                    nc.vector.tensor_copy(out=w3_sb[j], in_=zs)
                    nc.vector.tensor_copy(out=w3_sb[j][:, 8 * j:8 * j + 8],
                                          in_=w3c)

            def emit_qk_consts():
                for ci in range(2):
                    t = cst.tile([128, C], F32, tag=f"wq{ci}", name=f"wq{ci}")
                    nc.gpsimd.dma_start(out=t, in_=wq[128 * ci:128 * (ci + 1), :])
                    wq_sb.append(t)
                    t = cst.tile([128, C], F32, tag=f"wk{ci}", name=f"wk{ci}")
                    nc.gpsimd.dma_start(out=t, in_=wk[128 * ci:128 * (ci + 1), :])
                    wk_sb.append(t)
                    t = cst.tile([128, NP], F32, tag=f"cq{ci}", name=f"cq{ci}")
                    nc.gpsimd.dma_start(out=t, in_=ctq[128 * ci:128 * (ci + 1), :])
                    ctq_sb.append(t)
                    t = cst.tile([128, M], F32, tag=f"ck{ci}", name=f"ck{ci}")
                    nc.gpsimd.dma_start(out=t, in_=ctk[128 * ci:128 * (ci + 1), :])
                    ctk_sb.append(t)
                for nm, src in (("bq", bq), ("bk", bk)):
                    t = cst.tile([1, C], F32, tag=nm, name=nm)
                    nc.gpsimd.dma_start(out=t, in_=src[:, :])
                    misc[nm] = t
                t = cst.tile([1, M], F16, tag="mb", name="mbt")
                nc.gpsimd.dma_start(out=t, in_=mb[:, :])
                misc["mb"] = t
                t = cst.tile([1, 128], F32, tag="onc", name="onct")
                nc.vector.memset(t, 1.0)
                misc["onc"] = t

            def proj_T(dst, w_tiles, b_row, rhs_tiles, nfree, nm):
                # borrows one rotating psum tile
                pt = ppp.tile([128, 4, 512], F32, tag="T", name=f"proj{nm}")
                for ct in range(2):
                    p = pt[:, ct, 0:nfree]
                    for ci in range(2):
                        nc.tensor.matmul(
                            p,
                            w_tiles[ci][:, 128 * ct:128 * (ct + 1)],
                            rhs_tiles[ci][:, :nfree],
                            start=(ci == 0), stop=False)
                    nc.tensor.matmul(
                        p, b_row[:, 128 * ct:128 * (ct + 1)],
                        misc["onr"][:, :nfree], start=False, stop=True)
                    nc.vector.tensor_copy(out=dst[ct], in_=p)

            def emit_qt():
                proj_T(qt_sb, wq_sb, misc["bq"], ctq_sb, NP, "q")
                # pack Q into the per-c-half A_feat lhsT layout (block
                # sparse: head h only on its own 32 c-rows): zero-fill,
                # then one strided copy per head
                nc.vector.memset(qm_all.bitcast(F32), 0.0)
                for h in range(H):
                    a = 32 * (h % 4)
                    nc.vector.tensor_copy(
                        out=qm_all[a:a + 32, h // 4, :, :, h],
                        in_=qt_sb[h // 4][a:a + 32, :].rearrange(
                            "p (g q) -> p g q", g=NG))

            def emit_kt():
                proj_T(kt_sb, wk_sb, misc["bk"], ctk_sb, M, "k")

            def emit_v_consts():
                for ci in range(2):
                    t = cst.tile([128, C], F32, tag=f"wi{ci}", name=f"wi{ci}")
                    nc.gpsimd.dma_start(out=t, in_=win[128 * ci:128 * (ci + 1), :])
                    win_sb.append(t)
                t = cst.tile([1, C], F32, tag="bi", name="bi")
                nc.gpsimd.dma_start(out=t, in_=bin_[:, :])
                misc["bi"] = t

            def emit_v():
                pt = ppp.tile([128, 4, 512], F32, tag="T", name="projv")
                for t_, msz in MT:
                    p = pt[:, t_, 0:C]
                    for ci in range(2):
                        nc.tensor.matmul(
                            p[:msz, :],
                            ctk_sb[ci][:, 128 * t_:128 * t_ + msz],
                            win_sb[ci], start=(ci == 0), stop=False)
                    nc.tensor.matmul(
                        p[:msz, :], misc["onc"][:, :msz],
                        misc["bi"], start=False, stop=True)
                    nc.vector.tensor_copy(out=v_sb[t_][:msz, :],
                                          in_=p[:msz, :])

            def emit_tail_consts():
                t = cst.tile([128, 128], F16, tag="id", name="idt")
                nc.gpsimd.dma_start(out=t, in_=ident[:, :])
                misc["id"] = t
                for ci in range(2):
                    t = cst.tile([128, C], F32R, tag=f"wo{ci}", name=f"wo{ci}")
                    nc.gpsimd.dma_start(
                        out=t,
                        in_=wout[128 * ci:128 * (ci + 1), :].bitcast(F32R))
                    wout_sb.append(t)
                t = cst.tile([1, C], F32, tag="bo", name="bo")
                nc.gpsimd.dma_start(out=t, in_=bout[:, :])
                misc["bo"] = t

            # ---------------- P1 ----------------
            af_sb = big.tile([128, NG, M], F32, tag="af")
            acc_all = big.tile([128, NG, M], F32, tag="acc")
            pre_all = big.tile([128, NG, M], F32, tag="pre")
            x1x2 = {}
            acc_first = set()

            def emit_af(g, tgt):
                # A_feat for group g: full-M matmuls into bank 3 of the
                # NEXT step's tile (write->copy, then l2 overwrites)
                ps = tgt[:, 3, 0:M]
                nc.tensor.matmul(ps, qm_all[:, 0, g, :, :], kt_sb[0],
                                 start=True, stop=False)
                nc.tensor.matmul(ps, qm_all[:, 1, g, :, :], kt_sb[1],
                                 start=False, stop=False)
                nc.tensor.matmul(ps, onh[:, :128], misc["mb"],
                                 start=False, stop=True)
                nc.vector.tensor_copy(out=af_sb[:, g, :], in_=ps)

            def emit_silu3(q):
                # groups 4q..4q+4: swish(acc + b3), then + af -> pre
                al = smp.tile([128, 4, M], F32, tag="aloc", name=f"aloc{q}")
                nc.scalar.activation(out=al,
                                     in_=acc_all[:, 4 * q:4 * q + 4, :],
                                     func=AF.Silu, bias=b3_sb, scale=1.0)
                nc.vector.tensor_add(pre_all[:, 4 * q:4 * q + 4, :], al,
                                     af_sb[:, 4 * q:4 * q + 4, :])

            deferred = {
                0: [(emit_w3_dmas, None)],
                2: [(emit_qk_consts, None)],
                6: [(emit_qt, 0.012)],
                8: [(emit_kt, 0.014)],
                10: [(emit_v_consts, 0.016)],
                12: [(emit_v, 0.018)],
                14: [(emit_tail_consts, 0.020)],
            }
            af_at = {12 + 2 * g: g for g in range(NG)}
            s3_at = {}
            for q in range(4):
                last_row = 16 * (4 * q + 4) - 1
                for ci_, (r0, nr, hf) in enumerate(chunks):
                    if r0 <= last_row < r0 + nr and hf == 1:
                        s3_at[ci_ + 4] = q

            new_T(0)
            for s in range(NC + 4):
                for fn, gate in deferred.get(s, ()):
                    if gate is None:
                        fn()
                    else:
                        with tc.tile_wait_until(gate):
                            fn()

                ppa = T[s]
                if s < NC:
                    r0, nr, hf = chunks[s]
                    # prefetch one group ahead so l1 never stalls on the
                    # ~1.4us HWDGE latency at group boundaries
                    for g_need in (r0 // 16, (r0 + nr - 1) // 16,
                                   min(NG - 1, (r0 + nr - 1) // 16 + 1)):
                        if g_need not in gt_tiles:
                            need_gt(g_need)
                    # l1 rows into slots 0..6 (banks 0-1); b1 rides the
                    # gt ones-channel folded into w1.  Ragged chunks pad
                    # with duplicate rows so the merged AP never reads
                    # another rotation's stale bytes.
                    for j in range(6):
                        r = r0 + min(j, nr - 1)
                        nc.tensor.matmul(
                            slot(ppa, j),
                            w1_sb,
                            gt_tiles[r // 16][:, r % 16,
                                              MH * hf:MH * hf + MH],
                            start=True, stop=True)
                if 0 <= s - 2 < NC:
                    k = s - 2
                    r0k, nrk, hfk = chunks[k]
                    xk = x1x2[k]
                    # l2(k) into slots 6..12 (banks 2-3) + b2 bias matmul;
                    # padded rows duplicate row 0 (slots never left stale)
                    for j in range(6):
                        jj = min(j, nrk - 1)
                        nc.tensor.matmul(
                            slot(ppa, 6 + j),
                            w2_sb, xk[:, jj // 3, jj % 3, :],
                            start=True, stop=False)
                        nc.tensor.matmul(
                            slot(ppa, 6 + j),
                            b2_sb, onh[:, :MH],
                            start=False, stop=True)
                # merged silu: [l1(s) | l2(s-2)] in one 1632-col AP
                if s < NC + 2:
                    xt = xp.tile([128, 4, 3, MH], F16, tag="x", name=f"x{s}")
                    if s < NC and s - 2 >= 0:
                        nc.scalar.activation(
                            out=xt,
                            in_=ppa[:, :, 0:3 * MH].rearrange(
                                "p b (r m) -> p b r m", r=3),
                            func=AF.Silu, scale=1.0)
                    elif s < NC:
                        nc.scalar.activation(
                            out=xt[:, 0:2, :, :],
                            in_=ppa[:, 0:2, 0:3 * MH].rearrange(
                                "p b (r m) -> p b r m", r=3),
                            func=AF.Silu, scale=1.0)
                    else:
                        nc.scalar.activation(
                            out=xt[:, 2:4, :, :],
                            in_=ppa[:, 2:4, 0:3 * MH].rearrange(
                                "p b (r m) -> p b r m", r=3),
                            func=AF.Silu, scale=1.0)
                    x1x2[s] = xt
                # the NEXT step's tile: l3(s-4) and A_feat borrow regions
                # of it now (2 steps of data slack), then l1(s+1)/l2(s-1)
                # overwrite after the DVE reads.
                tnext = new_T(s + 1)
                if 0 <= s - 4 < NC:
                    k = s - 4
                    r0k, nrk, hfk = chunks[k]
                    runs = []
                    rcur = r0k
                    while rcur < r0k + nrk:
                        g = rcur // 16
                        rend = min(r0k + nrk, 16 * (g + 1))
                        runs.append((g, rcur, rend))
                        rcur = rend
                    for i, (g, ra, rb) in enumerate(runs):
                        l3ps = slot(tnext, 6 + i)
                        for jj, r in enumerate(range(ra, rb)):
                            j = r - r0k
                            nc.tensor.matmul(
                                l3ps, w3_sb[r % 16],
                                x1x2[k + 2][:, 2 + j // 3, j % 3, :],
                                start=(jj == 0), stop=(jj == rb - ra - 1))
                        dst = acc_all[:, g, MH * hfk:MH * hfk + MH]
                        if (g, hfk) not in acc_first:
                            acc_first.add((g, hfk))
                            nc.vector.tensor_copy(out=dst, in_=l3ps)
                        else:
                            nc.vector.tensor_add(dst, dst, l3ps)
                    x1x2.pop(k + 2, None)
                g_af = af_at.get(s)
                if g_af is not None:
                    emit_af(g_af, tnext)
                T.pop(s - 1, None)
                q3 = s3_at.get(s)
                if q3 is not None:
                    emit_silu3(q3)

            # ------------- P2: softmax + transpose; P3 per half ---------
            attT = big.tile([128, len(MT), 16, 16, 8], AV_DT, tag="attT")
            avf = smp.tile([128, 2, 256], F32R, tag="avf")

            def emit_av_half(hff):
                pvt = ppp.tile([128, 4, 512], F32, tag="T",
                               name=f"pav{hff}")
                pavs = [pvt[:, t, :].rearrange("p (a b) -> p a b", a=2)
                        for t, _ in MT]
                for h in range(H):
                    for t, msz in MT:
                        nc.tensor.matmul(
                            pavs[t][32 * (h % 4):32 * (h % 4) + 32, h // 4,
                                    128 * hff:128 * hff + 128],
                            v_sb[t][:msz, 32 * h:32 * h + 32],
                            attT[:msz, t, 8 * hff:8 * hff + 8, :, h],
                            start=True, stop=True,
                            tile_position=(0, 32 * (h % 4)))
                sl = np.s_[:, :, 128 * hff:128 * hff + 128]
                acc = smp.tile([128, 2, 128], F32, tag="avacc",
                               name=f"avacc{hff}")
                nc.vector.tensor_copy(out=acc, in_=pavs[0][sl])
                for t in range(1, len(MT) - 1):
                    nc.vector.tensor_add(acc, acc, pavs[t][sl])
                nc.vector.tensor_add(avf[sl], acc, pavs[len(MT) - 1][sl])
                po = pvt[:, 3, 0:C]
                for ci in range(2):
                    nc.tensor.matmul(
                        po,
                        avf[:, ci, 128 * hff:128 * hff + 128],
                        wout_sb[ci], start=(ci == 0), stop=False)
                nc.tensor.matmul(po, misc["onc"],
                                 misc["bo"], start=False, stop=True)
                o_sb = smp.tile([128, C], F32, tag="osb", name=f"osb{hff}")
                nc.vector.tensor_copy(out=o_sb, in_=po)
                nc.sync.dma_start(out=out[128 * hff:128 * (hff + 1), :],
                                  in_=o_sb)

            for gp in range(NG // 2):
                # f16 view of one bank of a rotating tile: two groups'
                # transposes share it so attT copies move [msz, 2, 128]
                # per instruction.
                pTt = ppp.tile([128, 4, 512], F32, tag="T", name=f"pT{gp}")
                pTv = pTt[:, 0, :].bitcast(F16).rearrange(
                    "p (a b) -> p a b", a=2)
                for j in range(2):
                    g = 2 * gp + j
                    att = smp.tile([128, M], F16, tag="att", name=f"att{g}")
                    sm = smp.tile([128, 1], F32, tag="sm", name=f"sm{g}")
                    nc.scalar.activation(out=att, in_=pre_all[:, g, :],
                                         func=AF.Exp, scale=1.0,
                                         accum_out=sm)
                    rc = smp.tile([128, 1], F32, tag="rc", name=f"rc{g}")
                    nc.vector.reciprocal(out=rc, in_=sm)
                    atts = smp.tile([128, M], F16, tag="atts",
                                    name=f"atts{g}")
                    eng = nc.vector if g >= NG - 2 else nc.gpsimd
                    eng.tensor_scalar_mul(atts, att, rc)
                    for t, msz in MT:
                        nc.tensor.transpose(
                            pTv[:msz, j, 128 * t:128 * t + 128],
                            atts[:, 128 * t:128 * t + msz],
                            misc["id"])
                for t, msz in MT:
                    nc.vector.tensor_copy(
                        out=attT[:msz, t, 2 * gp:2 * gp + 2, :, :],
                        in_=pTv[:msz, :, 128 * t:128 * t + 128])
                if gp == NG // 4 - 1:
                    emit_av_half(0)
            emit_av_half(1)

    nc.finalize()
    return nc


_CACHE = {}


def _get_nc(m_pad):
    if m_pad not in _CACHE:
        _CACHE[m_pad] = _build(bacc.Bacc(None, target_bir_lowering=False), m_pad)
    return _CACHE[m_pad]


def prepare(inputs):
    """Host-side sharding/packing. Returns (nc, in_maps, assemble)."""
    pg = np.asarray(inputs["pairwise_g"], np.float32)
    cf = np.asarray(inputs["coset_functions"], np.float32)
    mask = np.asarray(inputs["mask"])
    idxs = [np.where(mask[b])[0] for b in range(B)]
    maxc = max(len(ix) for ix in idxs)
    m_pad = max(256, -(-maxc // 16) * 16)

    lb1 = np.asarray(inputs["loc_b1"], np.float32).reshape(128)
    w1a = np.zeros((DG1, 128), np.float32)
    w1a[:DG] = np.asarray(inputs["loc_w1"], np.float32).transpose(1, 0, 2) \
        .reshape(DG, 128)
    w1a[DG] = lb1                       # b1 rides the gt ones-channel
    w2b = np.zeros((128, 128), np.float32)
    lw2 = np.asarray(inputs["loc_w2"], np.float32)
    for h in range(H):
        w2b[16 * h:16 * (h + 1), 16 * h:16 * (h + 1)] = lw2[h]
    lw3 = np.asarray(inputs["loc_w3"], np.float32)
    w3p = np.zeros((128, 8), np.float32)
    for h in range(H):
        w3p[16 * h:16 * (h + 1), h] = lw3[h, :, 0]
    b2v = np.asarray(inputs["loc_b2"], np.float32).reshape(1, 128)
    b3v = np.tile(np.asarray(inputs["loc_b3"], np.float32).reshape(8), 16)
    b3v = b3v.reshape(128, 1)

    common = {
        "wq": np.asarray(inputs["fc_q_w"], np.float32) / np.float32(16.0),
        "wk": np.asarray(inputs["fc_k_w"], np.float32),
        "win": np.asarray(inputs["in_w"], np.float32),
        "wout": np.asarray(inputs["out_w"], np.float32),
        "bq": (np.asarray(inputs["fc_q_b"], np.float32) / np.float32(16.0)
               ).reshape(1, C),
        "bk": np.asarray(inputs["fc_k_b"], np.float32).reshape(1, C),
        "bin": np.asarray(inputs["in_b"], np.float32).reshape(1, C),
        "bout": np.asarray(inputs["out_b"], np.float32).reshape(1, C),
        "w1": w1a.astype(np.float16), "w2": w2b.astype(np.float16),
        "w3": w3p.astype(np.float16),
        "b2": b2v.astype(np.float16), "b3": b3v,
        "ident": np.eye(128, dtype=np.float16),
    }
    common = {k: np.ascontiguousarray(v) for k, v in common.items()}

    in_maps = []
    for core in range(8):
        b, nh = core // 2, core % 2
        ix = idxs[b]
        cnt = len(ix)
        n0 = NP * nh
        gtc = np.zeros((DG1, NP, m_pad), np.float16)
        gtc[:DG, :, :cnt] = pg[b, n0:n0 + NP][:, ix, :].transpose(2, 0, 1)
        gtc[DG, :, :] = np.float16(1.0)   # ones-channel carrying b1
        ctk = np.zeros((C, m_pad), np.float32)
        ctk[:, :cnt] = cf[b, ix, :].T
        # pads at -100: exp underflows to 0 in the max-free softmax
        mbv = np.zeros((1, m_pad), np.float16)
        mbv[0, cnt:] = np.float16(-100.0)
        im = dict(common)
        im["gt"] = np.ascontiguousarray(gtc)
        im["ctq"] = np.ascontiguousarray(cf[b, n0:n0 + NP, :].T)
        im["ctk"] = np.ascontiguousarray(ctk)
        im["mb"] = mbv
        in_maps.append(im)

    def assemble(results):
        o = np.empty((B, N, C), np.float32)
        for core in range(8):
            b, nh = core // 2, core % 2
            o[b, NP * nh:NP * (nh + 1), :] = results[core]["out"]
        return o

    return _get_nc(m_pad), in_maps, assemble


def kernel(**inputs) -> np.ndarray:
    nc, in_maps, assemble = prepare(inputs)
    res = run_bass_kernel_spmd(nc, in_maps, list(range(8)))
    return assemble(res.results)
